# revision 20
# baseline (speedup 1.0000x reference)
"""Causal multi-head attention block (B=4, S=2048, NX=1024, H=16, D=64)
distributed over 8 TRN2 NeuronCores.

Sharding: core i handles batch b = i//2 and head-group hg = i%2 (8 of 16
heads).  Each core computes qkv for its heads, causal attention, and a
partial c_proj over its 512 feature rows; the per-batch pair of partials
is summed on the host while unsharding.

All matmuls run in bf16 (f32 PSUM accumulate).  Scores are computed in the
transposed orientation s^T[k, q] = k @ q^T.  The u = p @ v stage also runs
transposed: stationary = v_aug[k, (v | ones)] (ones half parity-swapped per
head), moving = the exp'd score tile p[k, q], accumulating u^T[d, q] in
PSUM with the softmax denominator replicated on the other 64 partitions.
This cuts the u matmuls to one wide-moving MM per (head, k-tile) with no
LDWEIGHTS churn, and writes a^T directly (no PE transpose pass).
finalize = DMA partition-shift of the denominator + reciprocal + one
tensor_tensor multiply per (head, q-chunk).
"""
import sys

sys.path.insert(0, "/opt/trn_rl_repo")

import functools

import ml_dtypes
import numpy as np

from concourse import bacc, mybir, tile
from concourse.bass_utils import run_bass_kernel_spmd

B, S, NX = 4, 2048, 1024
H, D = 16, 64
N_CORES = 8
HL = H // 2          # heads per core (local)
FL = HL * D          # local head feature width (512)
BF16 = mybir.dt.bfloat16
F32 = mybir.dt.float32
BF = ml_dtypes.bfloat16

NK = S // 128        # 16 k-tiles of 128
NQC = S // 512       # 4 q-chunks of 512
KK = NX // 128       # 8 contraction blocks

DEFAULT_CFG = "host-psw1024-psb3-pb8-pub2-nb-sc-xp-nq-gm"
DEFAULT_CFG_BIAS = "host-psw1024-psb3-pb8-pub2-sc-xp-nq"


def _parse_cfg(cfg: str):
    parts = cfg.split("-")
    d = {"mode": parts[0], "psw": 1536, "psb": 2, "pb": 8, "pub": 2,
         "nb": False, "sc": False, "do": False, "xp": False, "gm": False,
         "ac": False, "rsl": False, "nq": False}
    for p in parts[1:]:
        if p.startswith("psw"):
            d["psw"] = int(p[3:])
        elif p.startswith("psb"):
            d["psb"] = int(p[3:])
        elif p.startswith("pub"):
            d["pub"] = int(p[3:])
        elif p.startswith("pb"):
            d["pb"] = int(p[2:])
        elif p in d:
            d[p] = True
    return d


def _build(cfg: str):
    c = _parse_cfg(cfg)
    PSW, PSB, PB, PUB = c["psw"], c["psb"], c["pb"], c["pub"]
    NB, SC, DO, XP, GM, AC = (c["nb"], c["sc"], c["do"], c["xp"], c["gm"],
                              c["ac"])
    RSL = c["rsl"]
    NQ = c["nq"]
    GK = PSW // 512   # full k-tiles per exp group
    nc = bacc.Bacc("TRN2", target_bir_lowering=False, debug=False,
                   num_devices=N_CORES)

    xT_ext = nc.dram_tensor("xT", [NX, S], BF16, kind="ExternalInput")
    wqk_ext = nc.dram_tensor("w_qk", [NX, 2 * FL], BF16, kind="ExternalInput")
    wv_ext = nc.dram_tensor("w_v", [NX, FL], BF16, kind="ExternalInput")
    wp_ext = nc.dram_tensor("w_proj", [FL, NX], BF16, kind="ExternalInput")
    bqk_ext = nc.dram_tensor("b_qk", [2 * FL, 1], F32, kind="ExternalInput")
    bv_ext = nc.dram_tensor("bv_row", [1, FL], BF16, kind="ExternalInput")
    bp_ext = nc.dram_tensor("bp_row", [1, NX], BF16, kind="ExternalInput")
    out_ext = nc.dram_tensor("out", [S, NX], F32, kind="ExternalOutput")

    with tile.TileContext(nc) as tc:
        with tc.tile_pool(name="const", bufs=1) as cp, \
             tc.tile_pool(name="work", bufs=3) as wp, \
             tc.tile_pool(name="psS", bufs=PSB, space="PSUM") as psS, \
             tc.tile_pool(name="psU", bufs=PUB, space="PSUM") as psU:

            # ---- persistent SBUF tensors ----
            xT = cp.tile([128, KK, S], BF16, tag="xT")
            wqk = cp.tile([128, KK, 2 * FL], BF16, tag="wqk")
            wv = cp.tile([128, KK, FL], BF16, tag="wv")
            wproj = cp.tile([128, FL // 128, NX], BF16, tag="wproj")
            qkT = cp.tile([128, 2 * FL // 128, S], BF16, tag="qkT")
            # v_aug[k, kt, hh, par, 0:128]: per head pair hh, parity par:
            #   par=0 (even head): cols 0:64 = v, 64:128 = 1.0
            #   par=1 (odd head):  cols 0:64 = 1.0, 64:128 = v
            v5 = cp.tile([128, NK, HL // 2, 2, 128], BF16, tag="v5")
            aT = cp.tile([128, FL // 128, S], BF16, tag="aT")  # a^T [feat, q]
            bqk = cp.tile([128, 2 * FL // 128], F32, tag="bqk")
            bv_row = cp.tile([1, FL], BF16, tag="bv")
            bp_row = cp.tile([1, NX], BF16, tag="bp")
            ones_row = cp.tile([1, 128], BF16, tag="ones")
            tri = cp.tile([128, 128], BF16, tag="tri")
            triB = cp.tile([128, 128], BF16, tag="triB")

            # ---- input DMAs (ordered so compute can start early) ----
            # input DMAs spread across three engine rings (sync DMAs
            # serialize per ring at ~400GB/s); first-needed slices first:
            # qk(0)/qk(4) need wqk fb0/fb4 cols + xT chunk 0 only.
            wqk_src = wqk_ext.ap().rearrange("(kk p) f -> p kk f", p=128)
            xs = [xT_ext.ap()[:, sc * 512:(sc + 1) * 512]
                  .rearrange("(kk p) f -> p kk f", p=128) for sc in range(4)]
            nc.sync.dma_start(out=wqk[:, :, 0:128], in_=wqk_src[:, :, 0:128])
            nc.sync.dma_start(out=wqk[:, :, 512:640],
                              in_=wqk_src[:, :, 512:640])
            nc.scalar.dma_start(out=xT[:, :, 0:512], in_=xs[0])
            nc.gpsimd.dma_start(
                out=wv[:, :, :],
                in_=wv_ext.ap().rearrange("(kk p) f -> p kk f", p=128))
            nc.sync.dma_start(out=wqk[:, :, 128:512],
                              in_=wqk_src[:, :, 128:512])
            nc.sync.dma_start(out=wqk[:, :, 640:1024],
                              in_=wqk_src[:, :, 640:1024])
            nc.scalar.dma_start(out=xT[:, :, 512:1024], in_=xs[1])
            nc.gpsimd.dma_start(out=xT[:, :, 1024:1536], in_=xs[2])
            nc.scalar.dma_start(out=xT[:, :, 1536:2048], in_=xs[3])
            nc.sync.dma_start(
                out=wproj[:, :, :],
                in_=wp_ext.ap().rearrange("(kt p) f -> p kt f", p=128))
            nc.sync.dma_start(
                out=bqk[:, :],
                in_=bqk_ext.ap().rearrange("(fb p) o -> p (fb o)", p=128))
            nc.sync.dma_start(out=bv_row[:], in_=bv_ext.ap())
            nc.sync.dma_start(out=bp_row[:], in_=bp_ext.ap())

            nc.vector.memset(ones_row[:], 1.0)
            # tri[p, f] = 1 if p <= f else 0 (keep-in on p > f, else fill 1)
            nc.vector.memset(tri[:], 0.0)
            nc.gpsimd.affine_select(
                out=tri[:], in_=tri[:],
                compare_op=mybir.AluOpType.is_gt,
                fill=1.0, base=0, pattern=[[-1, 128]], channel_multiplier=1,
            )
            # row-swapped mask for odd heads' quadrant-score layout
            nc.vector.tensor_copy(triB[0:64, :], tri[64:128, :])
            nc.vector.tensor_copy(triB[64:128, :], tri[0:64, :])
            gm_zero = nc.gpsimd.to_reg(0.0) if GM else None
            # ones halves of v_aug (parity-swapped)
            nc.vector.memset(v5[:, :, :, 0, 64:128], 1.0)
            nc.vector.memset(v5[:, :, :, 1, 0:64], 1.0)

            # ---- stage 2: v (natural layout, split by head parity) ----
            def emit_v(st):
                ps = psS.tile([128, FL], F32, tag="ps")
                for kk in range(KK):
                    nc.tensor.matmul(ps[:], xT[:, kk, st * 128:(st + 1) * 128],
                                     wv[:, kk, :], start=(kk == 0),
                                     stop=(NB and kk == KK - 1))
                if not NB:
                    nc.tensor.matmul(ps[:], ones_row[:], bv_row[:],
                                     start=False, stop=True)
                ps_r = ps[:].rearrange("p (hh par d) -> p hh par d",
                                       par=2, d=D)
                nc.vector.tensor_copy(v5[:, st, :, 0, 0:D], ps_r[:, :, 0, :])
                if NQ:
                    nc.vector.tensor_copy(v5[:, st, :, 1, D:128],
                                          ps_r[:, :, 1, :])
                else:
                    # odd heads' p tiles have k-halves row-swapped
                    nc.vector.tensor_copy(v5[0:64, st, :, 1, D:128],
                                          ps_r[64:128, :, 1, :])
                    nc.vector.tensor_copy(v5[64:128, st, :, 1, D:128],
                                          ps_r[0:64, :, 1, :])

            # ---- stage 1: q^T / k^T (feature-major) ----
            qk_groups = ((0, 1536), (1536, 512)) if PSW >= 1536 else \
                        ((0, 1024), (1024, 1024))

            def emit_qk_half(fb, gi):
                for n0, nw in (qk_groups[gi],):
                    ps = psS.tile([128, nw], F32, tag="ps")
                    for c0 in range(0, nw, 512):
                        for kk in range(KK):
                            nc.tensor.matmul(
                                ps[:, c0:c0 + 512],
                                wqk[:, kk, fb * 128:(fb + 1) * 128],
                                xT[:, kk, n0 + c0:n0 + c0 + 512],
                                start=(kk == 0), stop=(kk == KK - 1))
                    if AC:
                        nc.scalar.activation(
                            qkT[:, fb, n0:n0 + nw], ps[:],
                            mybir.ActivationFunctionType.Identity,
                            bias=bqk[:, fb:fb + 1])
                    elif SC:
                        for s0 in range(0, nw, 512):
                            nc.vector.tensor_scalar_add(
                                qkT[:, fb, n0 + s0:n0 + s0 + 512],
                                ps[:, s0:s0 + 512], bqk[:, fb:fb + 1])
                    else:
                        nc.vector.tensor_scalar_add(qkT[:, fb, n0:n0 + nw],
                                                    ps[:], bqk[:, fb:fb + 1])

            # ---- stage 3: attention ----
            def head_ctx(lh, qc):
                n_full = 4 * qc
                groups = []
                kt0 = 0
                while kt0 < n_full:
                    g = min(GK, n_full - kt0)
                    groups.append([(kt0 + j, j * 512, 512, 0) for j in range(g)])
                    kt0 += g
                if PSW >= 1536:
                    diag_offs = (0, 512, 1024, 1280)
                    groups.append([(n_full + j, diag_offs[j], 512 - 128 * j,
                                    128 * j) for j in range(4)])
                else:
                    groups.append([(n_full + 0, 0, 512, 0),
                                   (n_full + 1, 512, 384, 128)])
                    groups.append([(n_full + 2, 0, 256, 256),
                                   (n_full + 3, 256, 128, 384)])
                return {"lh": lh, "fbq": lh // 2, "fbk": FL // 128 + lh // 2,
                        "po": (lh % 2) * 64, "qb": qc * 512, "qc": qc,
                        "n_full": n_full, "groups": groups,
                        "p": [None] * len(groups), "pu": None,
                        "last_kt": n_full + 3}

            def emit_scores_pair(A, Bc, gi):
                """Scores for both heads of a pair, one k-tile group.
                Quadrant mode (default): each k-tile becomes two rounds of
                two concurrent 64x64-tile matmuls — head A in array rows
                0-63, head B in rows 64-127, k-halves on opposite column
                groups — so both heads' scores stream in the time one head
                used to take.  Head B's PSUM k-halves come out row-swapped;
                its v tile and diag mask are pre-swapped to match."""
                g = A["groups"][gi]
                gw = max(off + N for (_, off, N, _) in g)
                psA = psS.tile([128, PSW], F32, tag="ps", name="psA")
                psB = psS.tile([128, PSW], F32, tag="ps", name="psB")
                pA = wp.tile([128, PSW], BF16, tag="p", bufs=PB, name="pA")
                pB = wp.tile([128, PSW], BF16, tag="p", bufs=PB, name="pB")
                for (kt, off, N, qoff) in g:
                    k0 = kt * 128
                    amv = qkT[0:64, A["fbq"], A["qb"] + qoff:A["qb"] + 512]
                    bmv = qkT[64:128, Bc["fbq"], Bc["qb"] + qoff:Bc["qb"] + 512]
                    if NQ:
                        nc.tensor.matmul(psA[:, off:off + N],
                                         qkT[0:64, A["fbk"], k0:k0 + 128],
                                         amv, start=True, stop=True)
                        nc.tensor.matmul(psB[:, off:off + N],
                                         qkT[64:128, Bc["fbk"], k0:k0 + 128],
                                         bmv, start=True, stop=True)
                    else:
                        nc.tensor.matmul(psA[0:64, off:off + N],
                                         qkT[0:64, A["fbk"], k0:k0 + 64],
                                         amv, start=True, stop=True)
                        nc.tensor.matmul(psB[64:128, off:off + N],
                                         qkT[64:128, Bc["fbk"], k0:k0 + 64],
                                         bmv, start=True, stop=True)
                        nc.tensor.matmul(psA[64:128, off:off + N],
                                         qkT[0:64, A["fbk"], k0 + 64:k0 + 128],
                                         amv, start=True, stop=True)
                        nc.tensor.matmul(psB[0:64, off:off + N],
                                         qkT[64:128, Bc["fbk"],
                                             k0 + 64:k0 + 128],
                                         bmv, start=True, stop=True)
                for ctx, ps, p in ((A, psA, pA), (Bc, psB, pB)):
                    nc.scalar.activation(p[:, 0:gw], ps[:, 0:gw],
                                         mybir.ActivationFunctionType.Exp,
                                         scale=0.125)
                    if g[0][0] >= A["n_full"]:
                        trit = tri if (NQ or ctx is A) else triB
                        for (kt, off, N, qoff) in g:
                            nc.vector.tensor_mul(p[:, off:off + 128],
                                                 p[:, off:off + 128], trit)
                    ctx["p"][gi] = p

            def emit_u(ctx, gi):
                if ctx["pu"] is None:
                    ctx["pu"] = psU.tile([128, 512], F32, tag="pu",
                                         name="pu_t")
                pu = ctx["pu"]
                p = ctx["p"][gi]
                lh = ctx["lh"]
                for (kt, off, N, qoff) in ctx["groups"][gi]:
                    nc.tensor.matmul(
                        pu[:, qoff:qoff + N],
                        v5[:, kt, lh >> 1, lh & 1, :],
                        p[:, off:off + N],
                        start=(kt == 0), stop=(kt == ctx["last_kt"]),
                        skip_group_check=True)

            def finalize(ctx):
                pu = ctx["pu"]
                po = ctx["po"]
                db = 64 - po             # denominator partitions
                rec = wp.tile([64, 512], F32, tag="rec", bufs=3, name="rec")
                if RSL:
                    nc.vector.reciprocal(rec[:, :], pu[db:db + 64, :])
                else:
                    # copy to SBUF base-0 first: reciprocal_approx_fast
                    # (custom DVE op) NaNs when fed PSUM directly
                    den = wp.tile([64, 512], F32, tag="den", bufs=3,
                                  name="den")
                    nc.vector.tensor_copy(den[:, :], pu[db:db + 64, :])
                    nc.vector.reciprocal_approx_fast(rec[:, :], den[:, :])
                nc.vector.tensor_mul(
                    aT[po:po + 64, ctx["fbq"], ctx["qb"]:ctx["qb"] + 512],
                    pu[po:po + 64, :], rec[:, :])

            pending = []

            def flush_pending():
                while pending:
                    pending.pop(0)()

            def emit_pair(pr, qc, fills=()):
                A = head_ctx(2 * pr, qc)
                Bc = head_ctx(2 * pr + 1, qc)
                fills = list(fills)
                n = len(A["groups"])
                emit_scores_pair(A, Bc, 0)
                # previous pair's tail (last u group + finalize) overlaps
                # this pair's first scores/exp instead of stalling the PE
                flush_pending()
                for i in range(n):
                    if i + 1 < n:
                        emit_scores_pair(A, Bc, i + 1)
                    if fills:
                        emit_unit(fills.pop(0))
                    if i < n - 1:
                        emit_u(A, i)
                        emit_u(Bc, i)
                    else:
                        def tail(Ax=A, Bx=Bc, gi=i):
                            emit_u(Ax, gi)
                            emit_u(Bx, gi)
                            finalize(Ax)
                            finalize(Bx)
                        pending.append(tail)
                for u in fills:
                    emit_unit(u)

            # ---- stage 4: c_proj partial from a^T ----
            def emit_proj(st):
                for n0 in range(0, NX, 512):
                    ps = psU.tile([128, 512], F32, tag="pu")
                    for kt in range(FL // 128):
                        nc.tensor.matmul(ps[:], aT[:, kt, st * 128:(st + 1) * 128],
                                         wproj[:, kt, n0:n0 + 512],
                                         start=(kt == 0),
                                         stop=(NB and kt == FL // 128 - 1))
                    if not NB:
                        nc.tensor.matmul(ps[:], ones_row[:],
                                         bp_row[:, n0:n0 + 512],
                                         start=False, stop=True)
                    dst = out_ext.ap()[st * 128:(st + 1) * 128, n0:n0 + 512]
                    if DO:
                        nc.sync.dma_start(out=dst, in_=ps[:])
                    else:
                        osb = wp.tile([128, 512], F32, tag="osb")
                        nc.vector.tensor_copy(osb[:], ps[:])
                        nc.sync.dma_start(out=dst, in_=osb[:])

            def emit_unit(u):
                kind = u[0]
                if kind == "v":
                    emit_v(u[1])
                elif kind == "qk":
                    emit_qk_half(u[1], u[2])
                else:
                    emit_proj(u[1])

            # ---- emission schedule: fully mixed stream ----
            # Pairs ordered so exp work reaches the scalar engine early
            # (it is the bottleneck in the attention phase); dense qkv/proj
            # units fill the PE while exps drain.  FB = FL//128 = 4.
            FB = FL // 128
            emit_qk_half(0, 0)
            emit_qk_half(FB, 0)
            for st in range(4):
                emit_v(st)
            emit_pair(0, 0, [("v", 4), ("v", 5)])
            emit_pair(0, 1, [("v", 6), ("v", 7), ("qk", 1, 0),
                             ("qk", FB + 1, 0)])
            emit_pair(1, 0, [("v", 8), ("v", 9)])
            emit_pair(1, 1, [("v", 10), ("v", 11), ("qk", 0, 1),
                             ("qk", FB, 1)])
            emit_pair(0, 2, [("qk", 2, 0), ("qk", FB + 2, 0), ("v", 12),
                             ("v", 13)])
            emit_pair(2, 0, [("v", 14), ("v", 15)])
            emit_pair(2, 1, [("qk", 3, 0), ("qk", FB + 3, 0), ("qk", 1, 1),
                             ("qk", FB + 1, 1)])
            emit_pair(1, 2)
            emit_pair(3, 0, [("qk", 2, 1), ("qk", FB + 2, 1)])
            emit_pair(3, 1, [("qk", 3, 1), ("qk", FB + 3, 1)])
            emit_pair(0, 3)
            emit_pair(2, 2, [("proj", 0), ("proj", 1)])
            emit_pair(1, 3, [("proj", 2), ("proj", 3)])
            emit_pair(3, 2, [("proj", 4), ("proj", 5)])
            emit_pair(2, 3, [("proj", 6), ("proj", 7)])
            emit_pair(3, 3, [("proj", 8), ("proj", 9), ("proj", 10),
                             ("proj", 11)])
            flush_pending()
            for st in range(12, NK):
                emit_proj(st)

    nc.compile()
    return nc


@functools.lru_cache(maxsize=2)
def _built(cfg: str):
    return _build(cfg)


def _in_maps(x, c_attn_w, c_attn_b, c_proj_w, c_proj_b):
    maps = []
    for core in range(N_CORES):
        b, hg = core // 2, core % 2
        f0 = hg * FL
        w_q = c_attn_w[:, f0:f0 + FL]
        w_k = c_attn_w[:, NX + f0:NX + f0 + FL]
        w_v = c_attn_w[:, 2 * NX + f0:2 * NX + f0 + FL]
        b_q = c_attn_b[f0:f0 + FL]
        b_k = c_attn_b[NX + f0:NX + f0 + FL]
        b_v = c_attn_b[2 * NX + f0:2 * NX + f0 + FL]
        maps.append({
            "xT": np.ascontiguousarray(x[b].T).astype(BF),
            "w_qk": np.concatenate([w_q, w_k], axis=1).astype(BF),
            "w_v": np.ascontiguousarray(w_v).astype(BF),
            "w_proj": np.ascontiguousarray(c_proj_w[f0:f0 + FL, :]).astype(BF),
            "b_qk": np.concatenate([b_q, b_k]).astype(np.float32).reshape(-1, 1),
            "bv_row": b_v.astype(BF).reshape(1, FL),
            "bp_row": (c_proj_b / 2.0).astype(BF).reshape(1, NX),
        })
    return maps


def _run(inputs, cfg=None, trace=False):
    if cfg is None:
        zero_bias = (not inputs["c_attn_b"].any()) and \
                    (not inputs["c_proj_b"].any())
        cfg = DEFAULT_CFG if zero_bias else DEFAULT_CFG_BIAS
    nc = _built(cfg)
    maps = _in_maps(inputs["x"], inputs["c_attn_w"], inputs["c_attn_b"],
                    inputs["c_proj_w"], inputs["c_proj_b"])
    res = run_bass_kernel_spmd(nc, maps, core_ids=list(range(N_CORES)),
                               trace=trace)
    out = np.empty((B, S, NX), dtype=np.float32)
    for b in range(B):
        out[b] = res.results[2 * b]["out"] + res.results[2 * b + 1]["out"]
    return out, res


def kernel(**inputs):
    out, _ = _run({k: np.asarray(v) for k, v in inputs.items()})
    return out


# revision 21
# speedup vs baseline: 1.0143x; 1.0143x over previous
"""Causal multi-head attention block (B=4, S=2048, NX=1024, H=16, D=64)
distributed over 8 TRN2 NeuronCores.

Sharding: core i handles batch b = i//2 and head-group hg = i%2 (8 of 16
heads).  Each core computes qkv for its heads, causal attention, and a
partial c_proj over its 512 feature rows; the per-batch pair of partials
is summed on the host while unsharding.

All matmuls run in bf16 (f32 PSUM accumulate).  Scores are computed in the
transposed orientation s^T[k, q] = k @ q^T.  The u = p @ v stage also runs
transposed: stationary = v_aug[k, (v | ones)] (ones half parity-swapped per
head), moving = the exp'd score tile p[k, q], accumulating u^T[d, q] in
PSUM with the softmax denominator replicated on the other 64 partitions.
This cuts the u matmuls to one wide-moving MM per (head, k-tile) with no
LDWEIGHTS churn, and writes a^T directly (no PE transpose pass).
finalize = DMA partition-shift of the denominator + reciprocal + one
tensor_tensor multiply per (head, q-chunk).
"""
import sys

sys.path.insert(0, "/opt/trn_rl_repo")

import functools

import ml_dtypes
import numpy as np

from concourse import bacc, mybir, tile
from concourse.bass_utils import run_bass_kernel_spmd

B, S, NX = 4, 2048, 1024
H, D = 16, 64
N_CORES = 8
HL = H // 2          # heads per core (local)
FL = HL * D          # local head feature width (512)
BF16 = mybir.dt.bfloat16
F32 = mybir.dt.float32
BF = ml_dtypes.bfloat16

NK = S // 128        # 16 k-tiles of 128
NQC = S // 512       # 4 q-chunks of 512
KK = NX // 128       # 8 contraction blocks

DEFAULT_CFG = "host-psw1024-psb3-pb8-pub2-nb-sc-xp-nq"
DEFAULT_CFG_BIAS = "host-psw1024-psb3-pb8-pub2-sc-xp-nq"


def _parse_cfg(cfg: str):
    parts = cfg.split("-")
    d = {"mode": parts[0], "psw": 1536, "psb": 2, "pb": 8, "pub": 2,
         "nb": False, "sc": False, "do": False, "xp": False, "gm": False,
         "ac": False, "rsl": False, "nq": False, "rfp": False}
    for p in parts[1:]:
        if p.startswith("psw"):
            d["psw"] = int(p[3:])
        elif p.startswith("psb"):
            d["psb"] = int(p[3:])
        elif p.startswith("pub"):
            d["pub"] = int(p[3:])
        elif p.startswith("pb"):
            d["pb"] = int(p[2:])
        elif p in d:
            d[p] = True
    return d


def _build(cfg: str):
    c = _parse_cfg(cfg)
    PSW, PSB, PB, PUB = c["psw"], c["psb"], c["pb"], c["pub"]
    NB, SC, DO, XP, GM, AC = (c["nb"], c["sc"], c["do"], c["xp"], c["gm"],
                              c["ac"])
    RSL = c["rsl"]
    NQ = c["nq"]
    RFP = c["rfp"]
    GK = PSW // 512   # full k-tiles per exp group
    nc = bacc.Bacc("TRN2", target_bir_lowering=False, debug=False,
                   num_devices=N_CORES)

    xT_ext = nc.dram_tensor("xT", [NX, S], BF16, kind="ExternalInput")
    wqk_ext = nc.dram_tensor("w_qk", [NX, 2 * FL], BF16, kind="ExternalInput")
    wv_ext = nc.dram_tensor("w_v", [NX, FL], BF16, kind="ExternalInput")
    wp_ext = nc.dram_tensor("w_proj", [FL, NX], BF16, kind="ExternalInput")
    bqk_ext = nc.dram_tensor("b_qk", [2 * FL, 1], F32, kind="ExternalInput")
    bv_ext = nc.dram_tensor("bv_row", [1, FL], BF16, kind="ExternalInput")
    bp_ext = nc.dram_tensor("bp_row", [1, NX], BF16, kind="ExternalInput")
    out_ext = nc.dram_tensor("out", [S, NX], F32, kind="ExternalOutput")

    with tile.TileContext(nc) as tc:
        with tc.tile_pool(name="const", bufs=1) as cp, \
             tc.tile_pool(name="work", bufs=3) as wp, \
             tc.tile_pool(name="psS", bufs=PSB, space="PSUM") as psS, \
             tc.tile_pool(name="psU", bufs=PUB, space="PSUM") as psU:

            # ---- persistent SBUF tensors ----
            xT = cp.tile([128, KK, S], BF16, tag="xT")
            wqk = cp.tile([128, KK, 2 * FL], BF16, tag="wqk")
            wv = cp.tile([128, KK, FL], BF16, tag="wv")
            wproj = cp.tile([128, FL // 128, NX], BF16, tag="wproj")
            qkT = cp.tile([128, 2 * FL // 128, S], BF16, tag="qkT")
            # v_aug[k, kt, hh, par, 0:128]: per head pair hh, parity par:
            #   par=0 (even head): cols 0:64 = v, 64:128 = 1.0
            #   par=1 (odd head):  cols 0:64 = 1.0, 64:128 = v
            v5 = cp.tile([128, NK, HL // 2, 2, 128], BF16, tag="v5")
            aT = cp.tile([128, FL // 128, S], BF16, tag="aT")  # a^T [feat, q]
            bqk = cp.tile([128, 2 * FL // 128], F32, tag="bqk")
            bv_row = cp.tile([1, FL], BF16, tag="bv")
            bp_row = cp.tile([1, NX], BF16, tag="bp")
            ones_row = cp.tile([1, 128], BF16, tag="ones")
            tri = cp.tile([128, 128], BF16, tag="tri")
            triB = cp.tile([128, 128], BF16, tag="triB")

            # ---- input DMAs (ordered so compute can start early) ----
            # input DMAs spread across three engine rings (sync DMAs
            # serialize per ring at ~400GB/s); first-needed slices first:
            # qk(0)/qk(4) need wqk fb0/fb4 cols + xT chunk 0 only.
            wqk_src = wqk_ext.ap().rearrange("(kk p) f -> p kk f", p=128)
            xs = [xT_ext.ap()[:, sc * 512:(sc + 1) * 512]
                  .rearrange("(kk p) f -> p kk f", p=128) for sc in range(4)]
            nc.sync.dma_start(out=wqk[:, :, 0:128], in_=wqk_src[:, :, 0:128])
            nc.sync.dma_start(out=wqk[:, :, 512:640],
                              in_=wqk_src[:, :, 512:640])
            nc.scalar.dma_start(out=xT[:, :, 0:512], in_=xs[0])
            nc.gpsimd.dma_start(
                out=wv[:, :, :],
                in_=wv_ext.ap().rearrange("(kk p) f -> p kk f", p=128))
            nc.scalar.dma_start(out=xT[:, :, 512:1024], in_=xs[1])
            nc.sync.dma_start(out=wqk[:, :, 128:512],
                              in_=wqk_src[:, :, 128:512])
            nc.gpsimd.dma_start(out=xT[:, :, 1024:1536], in_=xs[2])
            nc.sync.dma_start(out=wqk[:, :, 640:1024],
                              in_=wqk_src[:, :, 640:1024])
            nc.scalar.dma_start(out=xT[:, :, 1536:2048], in_=xs[3])
            nc.gpsimd.dma_start(
                out=wproj[:, :, :],
                in_=wp_ext.ap().rearrange("(kt p) f -> p kt f", p=128))
            nc.sync.dma_start(
                out=bqk[:, :],
                in_=bqk_ext.ap().rearrange("(fb p) o -> p (fb o)", p=128))
            nc.sync.dma_start(out=bv_row[:], in_=bv_ext.ap())
            nc.sync.dma_start(out=bp_row[:], in_=bp_ext.ap())

            nc.vector.memset(ones_row[:], 1.0)
            # tri[p, f] = 1 if p <= f else 0 (keep-in on p > f, else fill 1)
            nc.vector.memset(tri[:], 0.0)
            nc.gpsimd.affine_select(
                out=tri[:], in_=tri[:],
                compare_op=mybir.AluOpType.is_gt,
                fill=1.0, base=0, pattern=[[-1, 128]], channel_multiplier=1,
            )
            # row-swapped mask for odd heads' quadrant-score layout
            nc.vector.tensor_copy(triB[0:64, :], tri[64:128, :])
            nc.vector.tensor_copy(triB[64:128, :], tri[0:64, :])
            gm_zero = nc.gpsimd.to_reg(0.0) if GM else None
            # ones halves of v_aug (parity-swapped)
            nc.vector.memset(v5[:, :, :, 0, 64:128], 1.0)
            nc.vector.memset(v5[:, :, :, 1, 0:64], 1.0)

            # ---- stage 2: v (natural layout, split by head parity) ----
            def emit_v(st):
                ps = psS.tile([128, FL], F32, tag="ps")
                for kk in range(KK):
                    nc.tensor.matmul(ps[:], xT[:, kk, st * 128:(st + 1) * 128],
                                     wv[:, kk, :], start=(kk == 0),
                                     stop=(NB and kk == KK - 1))
                if not NB:
                    nc.tensor.matmul(ps[:], ones_row[:], bv_row[:],
                                     start=False, stop=True)
                ps_r = ps[:].rearrange("p (hh par d) -> p hh par d",
                                       par=2, d=D)
                nc.vector.tensor_copy(v5[:, st, :, 0, 0:D], ps_r[:, :, 0, :])
                if NQ:
                    nc.vector.tensor_copy(v5[:, st, :, 1, D:128],
                                          ps_r[:, :, 1, :])
                else:
                    # odd heads' p tiles have k-halves row-swapped
                    nc.vector.tensor_copy(v5[0:64, st, :, 1, D:128],
                                          ps_r[64:128, :, 1, :])
                    nc.vector.tensor_copy(v5[64:128, st, :, 1, D:128],
                                          ps_r[0:64, :, 1, :])

            # ---- stage 1: q^T / k^T (feature-major) ----
            qk_groups = ((0, 1536), (1536, 512)) if PSW >= 1536 else \
                        ((0, 1024), (1024, 1024))

            def emit_qk_half(fb, gi):
                for n0, nw in (qk_groups[gi],):
                    ps = psS.tile([128, nw], F32, tag="ps")
                    for c0 in range(0, nw, 512):
                        for kk in range(KK):
                            nc.tensor.matmul(
                                ps[:, c0:c0 + 512],
                                wqk[:, kk, fb * 128:(fb + 1) * 128],
                                xT[:, kk, n0 + c0:n0 + c0 + 512],
                                start=(kk == 0), stop=(kk == KK - 1))
                    if AC:
                        nc.scalar.activation(
                            qkT[:, fb, n0:n0 + nw], ps[:],
                            mybir.ActivationFunctionType.Identity,
                            bias=bqk[:, fb:fb + 1])
                    elif SC:
                        for s0 in range(0, nw, 512):
                            nc.vector.tensor_scalar_add(
                                qkT[:, fb, n0 + s0:n0 + s0 + 512],
                                ps[:, s0:s0 + 512], bqk[:, fb:fb + 1])
                    else:
                        nc.vector.tensor_scalar_add(qkT[:, fb, n0:n0 + nw],
                                                    ps[:], bqk[:, fb:fb + 1])

            # ---- stage 3: attention ----
            def head_ctx(lh, qc):
                n_full = 4 * qc
                groups = []
                kt0 = 0
                while kt0 < n_full:
                    g = min(GK, n_full - kt0)
                    groups.append([(kt0 + j, j * 512, 512, 0) for j in range(g)])
                    kt0 += g
                if PSW >= 1536:
                    diag_offs = (0, 512, 1024, 1280)
                    groups.append([(n_full + j, diag_offs[j], 512 - 128 * j,
                                    128 * j) for j in range(4)])
                else:
                    groups.append([(n_full + 0, 0, 512, 0),
                                   (n_full + 1, 512, 384, 128)])
                    groups.append([(n_full + 2, 0, 256, 256),
                                   (n_full + 3, 256, 128, 384)])
                return {"lh": lh, "fbq": lh // 2, "fbk": FL // 128 + lh // 2,
                        "po": (lh % 2) * 64, "qb": qc * 512, "qc": qc,
                        "n_full": n_full, "groups": groups,
                        "p": [None] * len(groups), "pu": None,
                        "last_kt": n_full + 3}

            def emit_scores_pair(A, Bc, gi):
                """Scores for both heads of a pair, one k-tile group.
                Quadrant mode (default): each k-tile becomes two rounds of
                two concurrent 64x64-tile matmuls — head A in array rows
                0-63, head B in rows 64-127, k-halves on opposite column
                groups — so both heads' scores stream in the time one head
                used to take.  Head B's PSUM k-halves come out row-swapped;
                its v tile and diag mask are pre-swapped to match."""
                g = A["groups"][gi]
                gw = max(off + N for (_, off, N, _) in g)
                psA = psS.tile([128, PSW], F32, tag="ps", name="psA")
                psB = psS.tile([128, PSW], F32, tag="ps", name="psB")
                pA = wp.tile([128, PSW], BF16, tag="p", bufs=PB, name="pA")
                pB = wp.tile([128, PSW], BF16, tag="p", bufs=PB, name="pB")
                for (kt, off, N, qoff) in g:
                    k0 = kt * 128
                    amv = qkT[0:64, A["fbq"], A["qb"] + qoff:A["qb"] + 512]
                    bmv = qkT[64:128, Bc["fbq"], Bc["qb"] + qoff:Bc["qb"] + 512]
                    if NQ:
                        nc.tensor.matmul(psA[:, off:off + N],
                                         qkT[0:64, A["fbk"], k0:k0 + 128],
                                         amv, start=True, stop=True)
                        nc.tensor.matmul(psB[:, off:off + N],
                                         qkT[64:128, Bc["fbk"], k0:k0 + 128],
                                         bmv, start=True, stop=True)
                    else:
                        nc.tensor.matmul(psA[0:64, off:off + N],
                                         qkT[0:64, A["fbk"], k0:k0 + 64],
                                         amv, start=True, stop=True)
                        nc.tensor.matmul(psB[64:128, off:off + N],
                                         qkT[64:128, Bc["fbk"], k0:k0 + 64],
                                         bmv, start=True, stop=True)
                        nc.tensor.matmul(psA[64:128, off:off + N],
                                         qkT[0:64, A["fbk"], k0 + 64:k0 + 128],
                                         amv, start=True, stop=True)
                        nc.tensor.matmul(psB[0:64, off:off + N],
                                         qkT[64:128, Bc["fbk"],
                                             k0 + 64:k0 + 128],
                                         bmv, start=True, stop=True)
                for ctx, ps, p in ((A, psA, pA), (Bc, psB, pB)):
                    nc.scalar.activation(p[:, 0:gw], ps[:, 0:gw],
                                         mybir.ActivationFunctionType.Exp,
                                         scale=0.125)
                    if g[0][0] >= A["n_full"]:
                        trit = tri if (NQ or ctx is A) else triB
                        for (kt, off, N, qoff) in g:
                            nc.vector.tensor_mul(p[:, off:off + 128],
                                                 p[:, off:off + 128], trit)
                    ctx["p"][gi] = p

            def emit_u(ctx, gi):
                if ctx["pu"] is None:
                    ctx["pu"] = psU.tile([128, 512], F32, tag="pu",
                                         name="pu_t")
                pu = ctx["pu"]
                p = ctx["p"][gi]
                lh = ctx["lh"]
                for (kt, off, N, qoff) in ctx["groups"][gi]:
                    nc.tensor.matmul(
                        pu[:, qoff:qoff + N],
                        v5[:, kt, lh >> 1, lh & 1, :],
                        p[:, off:off + N],
                        start=(kt == 0), stop=(kt == ctx["last_kt"]),
                        skip_group_check=True)

            def finalize(ctx):
                pu = ctx["pu"]
                po = ctx["po"]
                db = 64 - po             # denominator partitions
                rec = wp.tile([64, 512], F32, tag="rec", bufs=3, name="rec")
                if RSL:
                    nc.vector.reciprocal(rec[:, :], pu[db:db + 64, :])
                elif RFP:
                    nc.vector.reciprocal_approx_fast(rec[:, :],
                                                     pu[db:db + 64, :])
                else:
                    # copy to SBUF base-0 first: reciprocal_approx_fast
                    # (custom DVE op) NaNs when fed PSUM directly
                    den = wp.tile([64, 512], F32, tag="den", bufs=3,
                                  name="den")
                    nc.vector.tensor_copy(den[:, :], pu[db:db + 64, :])
                    nc.vector.reciprocal_approx_fast(rec[:, :], den[:, :])
                nc.vector.tensor_mul(
                    aT[po:po + 64, ctx["fbq"], ctx["qb"]:ctx["qb"] + 512],
                    pu[po:po + 64, :], rec[:, :])

            pending = []

            def flush_pending():
                while pending:
                    pending.pop(0)()

            def emit_pair(pr, qc, fills=()):
                A = head_ctx(2 * pr, qc)
                Bc = head_ctx(2 * pr + 1, qc)
                fills = list(fills)
                n = len(A["groups"])
                emit_scores_pair(A, Bc, 0)
                # previous pair's tail (last u group + finalize) overlaps
                # this pair's first scores/exp instead of stalling the PE
                flush_pending()
                for i in range(n):
                    if i + 1 < n:
                        emit_scores_pair(A, Bc, i + 1)
                    if fills:
                        emit_unit(fills.pop(0))
                    if i < n - 1:
                        emit_u(A, i)
                        emit_u(Bc, i)
                    else:
                        def tail(Ax=A, Bx=Bc, gi=i):
                            emit_u(Ax, gi)
                            emit_u(Bx, gi)
                            finalize(Ax)
                            finalize(Bx)
                        pending.append(tail)
                for u in fills:
                    emit_unit(u)

            # ---- stage 4: c_proj partial from a^T ----
            def emit_proj(st):
                for n0 in range(0, NX, 512):
                    ps = psU.tile([128, 512], F32, tag="pu")
                    for kt in range(FL // 128):
                        nc.tensor.matmul(ps[:], aT[:, kt, st * 128:(st + 1) * 128],
                                         wproj[:, kt, n0:n0 + 512],
                                         start=(kt == 0),
                                         stop=(NB and kt == FL // 128 - 1))
                    if not NB:
                        nc.tensor.matmul(ps[:], ones_row[:],
                                         bp_row[:, n0:n0 + 512],
                                         start=False, stop=True)
                    dst = out_ext.ap()[st * 128:(st + 1) * 128, n0:n0 + 512]
                    osb = wp.tile([128, 512], F32, tag="osb")
                    if st >= 12:
                        nc.scalar.copy(osb[:], ps[:])
                    else:
                        nc.vector.tensor_copy(osb[:], ps[:])
                    nc.sync.dma_start(out=dst, in_=osb[:])

            def emit_unit(u):
                kind = u[0]
                if kind == "v":
                    emit_v(u[1])
                elif kind == "qk":
                    emit_qk_half(u[1], u[2])
                else:
                    emit_proj(u[1])

            # ---- emission schedule: fully mixed stream ----
            # Pairs ordered so exp work reaches the scalar engine early
            # (it is the bottleneck in the attention phase); dense qkv/proj
            # units fill the PE while exps drain.  FB = FL//128 = 4.
            FB = FL // 128
            emit_qk_half(0, 0)
            emit_qk_half(FB, 0)
            for st in range(4):
                emit_v(st)
            emit_pair(0, 0, [("v", 4), ("v", 5)])
            emit_pair(0, 1, [("v", 6), ("v", 7), ("qk", 1, 0),
                             ("qk", FB + 1, 0)])
            emit_pair(1, 0, [("v", 8), ("v", 9)])
            emit_pair(1, 1, [("v", 10), ("v", 11), ("qk", 0, 1),
                             ("qk", FB, 1)])
            emit_pair(0, 2, [("qk", 2, 0), ("qk", FB + 2, 0), ("v", 12),
                             ("v", 13)])
            emit_pair(2, 0, [("v", 14), ("v", 15)])
            emit_pair(2, 1, [("qk", 3, 0), ("qk", FB + 3, 0), ("qk", 1, 1),
                             ("qk", FB + 1, 1)])
            emit_pair(1, 2)
            emit_pair(3, 0, [("qk", 2, 1), ("qk", FB + 2, 1)])
            emit_pair(3, 1, [("qk", 3, 1), ("qk", FB + 3, 1)])
            emit_pair(0, 3)
            emit_pair(2, 2, [("proj", 0), ("proj", 1)])
            emit_pair(1, 3, [("proj", 2), ("proj", 3)])
            emit_pair(3, 2, [("proj", 4), ("proj", 5)])
            emit_pair(2, 3, [("proj", 6), ("proj", 7)])
            emit_pair(3, 3, [("proj", 8), ("proj", 9), ("proj", 10),
                             ("proj", 11)])
            flush_pending()
            for st in range(12, NK):
                emit_proj(st)

    nc.compile()
    return nc


@functools.lru_cache(maxsize=2)
def _built(cfg: str):
    return _build(cfg)


def _in_maps(x, c_attn_w, c_attn_b, c_proj_w, c_proj_b):
    maps = []
    for core in range(N_CORES):
        b, hg = core // 2, core % 2
        f0 = hg * FL
        w_q = c_attn_w[:, f0:f0 + FL]
        w_k = c_attn_w[:, NX + f0:NX + f0 + FL]
        w_v = c_attn_w[:, 2 * NX + f0:2 * NX + f0 + FL]
        b_q = c_attn_b[f0:f0 + FL]
        b_k = c_attn_b[NX + f0:NX + f0 + FL]
        b_v = c_attn_b[2 * NX + f0:2 * NX + f0 + FL]
        maps.append({
            "xT": np.ascontiguousarray(x[b].T).astype(BF),
            "w_qk": np.concatenate([w_q, w_k], axis=1).astype(BF),
            "w_v": np.ascontiguousarray(w_v).astype(BF),
            "w_proj": np.ascontiguousarray(c_proj_w[f0:f0 + FL, :]).astype(BF),
            "b_qk": np.concatenate([b_q, b_k]).astype(np.float32).reshape(-1, 1),
            "bv_row": b_v.astype(BF).reshape(1, FL),
            "bp_row": (c_proj_b / 2.0).astype(BF).reshape(1, NX),
        })
    return maps


def _run(inputs, cfg=None, trace=False):
    if cfg is None:
        zero_bias = (not inputs["c_attn_b"].any()) and \
                    (not inputs["c_proj_b"].any())
        cfg = DEFAULT_CFG if zero_bias else DEFAULT_CFG_BIAS
    nc = _built(cfg)
    maps = _in_maps(inputs["x"], inputs["c_attn_w"], inputs["c_attn_b"],
                    inputs["c_proj_w"], inputs["c_proj_b"])
    res = run_bass_kernel_spmd(nc, maps, core_ids=list(range(N_CORES)),
                               trace=trace)
    out = np.empty((B, S, NX), dtype=np.float32)
    for b in range(B):
        out[b] = res.results[2 * b]["out"] + res.results[2 * b + 1]["out"]
    return out, res


def kernel(**inputs):
    out, _ = _run({k: np.asarray(v) for k, v in inputs.items()})
    return out


# revision 23
# speedup vs baseline: 1.0158x; 1.0015x over previous
"""Causal multi-head attention block (B=4, S=2048, NX=1024, H=16, D=64)
distributed over 8 TRN2 NeuronCores.

Sharding: core i handles batch b = i//2 and head-group hg = i%2 (8 of 16
heads).  Each core computes qkv for its heads, causal attention, and a
partial c_proj over its 512 feature rows; the per-batch pair of partials
is summed on the host while unsharding.

All matmuls run in bf16 (f32 PSUM accumulate).  Scores are computed in the
transposed orientation s^T[k, q] = k @ q^T.  The u = p @ v stage also runs
transposed: stationary = v_aug[k, (v | ones)] (ones half parity-swapped per
head), moving = the exp'd score tile p[k, q], accumulating u^T[d, q] in
PSUM with the softmax denominator replicated on the other 64 partitions.
This cuts the u matmuls to one wide-moving MM per (head, k-tile) with no
LDWEIGHTS churn, and writes a^T directly (no PE transpose pass).
finalize = DMA partition-shift of the denominator + reciprocal + one
tensor_tensor multiply per (head, q-chunk).
"""
import sys

sys.path.insert(0, "/opt/trn_rl_repo")

import functools

import ml_dtypes
import numpy as np

from concourse import bacc, mybir, tile
from concourse.bass_utils import run_bass_kernel_spmd

B, S, NX = 4, 2048, 1024
H, D = 16, 64
N_CORES = 8
HL = H // 2          # heads per core (local)
FL = HL * D          # local head feature width (512)
BF16 = mybir.dt.bfloat16
F32 = mybir.dt.float32
BF = ml_dtypes.bfloat16

NK = S // 128        # 16 k-tiles of 128
NQC = S // 512       # 4 q-chunks of 512
KK = NX // 128       # 8 contraction blocks

DEFAULT_CFG = "host-psw1024-psb3-pb8-pub2-nb-sc-xp-nq"
DEFAULT_CFG_BIAS = "host-psw1024-psb3-pb8-pub2-sc-xp-nq"


def _parse_cfg(cfg: str):
    parts = cfg.split("-")
    d = {"mode": parts[0], "psw": 1536, "psb": 2, "pb": 8, "pub": 2,
         "nb": False, "sc": False, "do": False, "xp": False, "gm": False,
         "ac": False, "rsl": False, "nq": False, "rfp": False}
    for p in parts[1:]:
        if p.startswith("psw"):
            d["psw"] = int(p[3:])
        elif p.startswith("psb"):
            d["psb"] = int(p[3:])
        elif p.startswith("pub"):
            d["pub"] = int(p[3:])
        elif p.startswith("pb"):
            d["pb"] = int(p[2:])
        elif p in d:
            d[p] = True
    return d


def _build(cfg: str):
    c = _parse_cfg(cfg)
    PSW, PSB, PB, PUB = c["psw"], c["psb"], c["pb"], c["pub"]
    NB, SC, DO, XP, GM, AC = (c["nb"], c["sc"], c["do"], c["xp"], c["gm"],
                              c["ac"])
    RSL = c["rsl"]
    NQ = c["nq"]
    RFP = c["rfp"]
    GK = PSW // 512   # full k-tiles per exp group
    nc = bacc.Bacc("TRN2", target_bir_lowering=False, debug=False,
                   num_devices=N_CORES)

    xT_ext = nc.dram_tensor("xT", [NX, S], BF16, kind="ExternalInput")
    wqk_ext = nc.dram_tensor("w_qk", [NX, 2 * FL], BF16, kind="ExternalInput")
    wv_ext = nc.dram_tensor("w_v", [NX, FL], BF16, kind="ExternalInput")
    wp_ext = nc.dram_tensor("w_proj", [FL, NX], BF16, kind="ExternalInput")
    bqk_ext = nc.dram_tensor("b_qk", [2 * FL, 1], F32, kind="ExternalInput")
    bv_ext = nc.dram_tensor("bv_row", [1, FL], BF16, kind="ExternalInput")
    bp_ext = nc.dram_tensor("bp_row", [1, NX], BF16, kind="ExternalInput")
    out_ext = nc.dram_tensor("out", [S, NX], F32, kind="ExternalOutput")

    with tile.TileContext(nc) as tc:
        with tc.tile_pool(name="const", bufs=1) as cp, \
             tc.tile_pool(name="work", bufs=3) as wp, \
             tc.tile_pool(name="psS", bufs=PSB, space="PSUM") as psS, \
             tc.tile_pool(name="psU", bufs=PUB, space="PSUM") as psU:

            # ---- persistent SBUF tensors ----
            xT = cp.tile([128, KK, S], BF16, tag="xT")
            wqk = cp.tile([128, KK, 2 * FL], BF16, tag="wqk")
            wv = cp.tile([128, KK, FL], BF16, tag="wv")
            wproj = cp.tile([128, FL // 128, NX], BF16, tag="wproj")
            qkT = cp.tile([128, 2 * FL // 128, S], BF16, tag="qkT")
            # v_aug[k, kt, hh, par, 0:128]: per head pair hh, parity par:
            #   par=0 (even head): cols 0:64 = v, 64:128 = 1.0
            #   par=1 (odd head):  cols 0:64 = 1.0, 64:128 = v
            v5 = cp.tile([128, NK, HL // 2, 2, 128], BF16, tag="v5")
            aT = cp.tile([128, FL // 128, S], BF16, tag="aT")  # a^T [feat, q]
            bqk = cp.tile([128, 2 * FL // 128], F32, tag="bqk")
            bv_row = cp.tile([1, FL], BF16, tag="bv")
            bp_row = cp.tile([1, NX], BF16, tag="bp")
            ones_row = cp.tile([1, 128], BF16, tag="ones")
            tri = cp.tile([128, 128], BF16, tag="tri")
            triB = cp.tile([128, 128], BF16, tag="triB")

            # ---- input DMAs (ordered so compute can start early) ----
            # input DMAs spread across three engine rings (sync DMAs
            # serialize per ring at ~400GB/s); first-needed slices first:
            # qk(0)/qk(4) need wqk fb0/fb4 cols + xT chunk 0 only.
            wqk_src = wqk_ext.ap().rearrange("(kk p) f -> p kk f", p=128)
            xs = [xT_ext.ap()[:, sc * 512:(sc + 1) * 512]
                  .rearrange("(kk p) f -> p kk f", p=128) for sc in range(4)]
            # full-width row slices keep 2KB contiguous segments (fast);
            # kk-halves go on separate rings for parallelism
            nc.sync.dma_start(out=xT[:, :, 0:512], in_=xs[0])
            nc.scalar.dma_start(out=wqk[:, 0:4, :], in_=wqk_src[:, 0:4, :])
            nc.gpsimd.dma_start(out=wqk[:, 4:8, :], in_=wqk_src[:, 4:8, :])
            nc.sync.dma_start(
                out=wv[:, :, :],
                in_=wv_ext.ap().rearrange("(kk p) f -> p kk f", p=128))
            nc.scalar.dma_start(out=xT[:, :, 512:1024], in_=xs[1])
            nc.gpsimd.dma_start(out=xT[:, :, 1024:1536], in_=xs[2])
            nc.sync.dma_start(out=xT[:, :, 1536:2048], in_=xs[3])
            nc.scalar.dma_start(
                out=wproj[:, :, :],
                in_=wp_ext.ap().rearrange("(kt p) f -> p kt f", p=128))
            nc.sync.dma_start(
                out=bqk[:, :],
                in_=bqk_ext.ap().rearrange("(fb p) o -> p (fb o)", p=128))
            nc.sync.dma_start(out=bv_row[:], in_=bv_ext.ap())
            nc.sync.dma_start(out=bp_row[:], in_=bp_ext.ap())

            nc.vector.memset(ones_row[:], 1.0)
            # tri[p, f] = 1 if p <= f else 0 (keep-in on p > f, else fill 1)
            nc.vector.memset(tri[:], 0.0)
            nc.gpsimd.affine_select(
                out=tri[:], in_=tri[:],
                compare_op=mybir.AluOpType.is_gt,
                fill=1.0, base=0, pattern=[[-1, 128]], channel_multiplier=1,
            )
            # row-swapped mask for odd heads' quadrant-score layout
            nc.vector.tensor_copy(triB[0:64, :], tri[64:128, :])
            nc.vector.tensor_copy(triB[64:128, :], tri[0:64, :])
            gm_zero = nc.gpsimd.to_reg(0.0) if GM else None
            # ones halves of v_aug (parity-swapped)
            nc.vector.memset(v5[:, :, :, 0, 64:128], 1.0)
            nc.vector.memset(v5[:, :, :, 1, 0:64], 1.0)

            # ---- stage 2: v (natural layout, split by head parity) ----
            def emit_v(st):
                ps = psS.tile([128, FL], F32, tag="ps")
                for kk in range(KK):
                    nc.tensor.matmul(ps[:], xT[:, kk, st * 128:(st + 1) * 128],
                                     wv[:, kk, :], start=(kk == 0),
                                     stop=(NB and kk == KK - 1))
                if not NB:
                    nc.tensor.matmul(ps[:], ones_row[:], bv_row[:],
                                     start=False, stop=True)
                ps_r = ps[:].rearrange("p (hh par d) -> p hh par d",
                                       par=2, d=D)
                nc.vector.tensor_copy(v5[:, st, :, 0, 0:D], ps_r[:, :, 0, :])
                if NQ:
                    nc.vector.tensor_copy(v5[:, st, :, 1, D:128],
                                          ps_r[:, :, 1, :])
                else:
                    # odd heads' p tiles have k-halves row-swapped
                    nc.vector.tensor_copy(v5[0:64, st, :, 1, D:128],
                                          ps_r[64:128, :, 1, :])
                    nc.vector.tensor_copy(v5[64:128, st, :, 1, D:128],
                                          ps_r[0:64, :, 1, :])

            # ---- stage 1: q^T / k^T (feature-major) ----
            qk_groups = ((0, 1536), (1536, 512)) if PSW >= 1536 else \
                        ((0, 1024), (1024, 1024))

            def emit_qk_half(fb, gi):
                for n0, nw in (qk_groups[gi],):
                    ps = psS.tile([128, nw], F32, tag="ps")
                    for c0 in range(0, nw, 512):
                        for kk in range(KK):
                            nc.tensor.matmul(
                                ps[:, c0:c0 + 512],
                                wqk[:, kk, fb * 128:(fb + 1) * 128],
                                xT[:, kk, n0 + c0:n0 + c0 + 512],
                                start=(kk == 0), stop=(kk == KK - 1))
                    if AC:
                        nc.scalar.activation(
                            qkT[:, fb, n0:n0 + nw], ps[:],
                            mybir.ActivationFunctionType.Identity,
                            bias=bqk[:, fb:fb + 1])
                    elif SC:
                        for s0 in range(0, nw, 512):
                            nc.vector.tensor_scalar_add(
                                qkT[:, fb, n0 + s0:n0 + s0 + 512],
                                ps[:, s0:s0 + 512], bqk[:, fb:fb + 1])
                    else:
                        nc.vector.tensor_scalar_add(qkT[:, fb, n0:n0 + nw],
                                                    ps[:], bqk[:, fb:fb + 1])

            # ---- stage 3: attention ----
            def head_ctx(lh, qc):
                n_full = 4 * qc
                groups = []
                kt0 = 0
                while kt0 < n_full:
                    g = min(GK, n_full - kt0)
                    groups.append([(kt0 + j, j * 512, 512, 0) for j in range(g)])
                    kt0 += g
                if PSW >= 1536:
                    diag_offs = (0, 512, 1024, 1280)
                    groups.append([(n_full + j, diag_offs[j], 512 - 128 * j,
                                    128 * j) for j in range(4)])
                else:
                    groups.append([(n_full + 0, 0, 512, 0),
                                   (n_full + 1, 512, 384, 128)])
                    groups.append([(n_full + 2, 0, 256, 256),
                                   (n_full + 3, 256, 128, 384)])
                return {"lh": lh, "fbq": lh // 2, "fbk": FL // 128 + lh // 2,
                        "po": (lh % 2) * 64, "qb": qc * 512, "qc": qc,
                        "n_full": n_full, "groups": groups,
                        "p": [None] * len(groups), "pu": None,
                        "last_kt": n_full + 3}

            def emit_scores_pair(A, Bc, gi):
                """Scores for both heads of a pair, one k-tile group.
                Quadrant mode (default): each k-tile becomes two rounds of
                two concurrent 64x64-tile matmuls — head A in array rows
                0-63, head B in rows 64-127, k-halves on opposite column
                groups — so both heads' scores stream in the time one head
                used to take.  Head B's PSUM k-halves come out row-swapped;
                its v tile and diag mask are pre-swapped to match."""
                g = A["groups"][gi]
                gw = max(off + N for (_, off, N, _) in g)
                psA = psS.tile([128, PSW], F32, tag="ps", name="psA")
                psB = psS.tile([128, PSW], F32, tag="ps", name="psB")
                pA = wp.tile([128, PSW], BF16, tag="p", bufs=PB, name="pA")
                pB = wp.tile([128, PSW], BF16, tag="p", bufs=PB, name="pB")
                for (kt, off, N, qoff) in g:
                    k0 = kt * 128
                    amv = qkT[0:64, A["fbq"], A["qb"] + qoff:A["qb"] + 512]
                    bmv = qkT[64:128, Bc["fbq"], Bc["qb"] + qoff:Bc["qb"] + 512]
                    if NQ:
                        nc.tensor.matmul(psA[:, off:off + N],
                                         qkT[0:64, A["fbk"], k0:k0 + 128],
                                         amv, start=True, stop=True)
                        nc.tensor.matmul(psB[:, off:off + N],
                                         qkT[64:128, Bc["fbk"], k0:k0 + 128],
                                         bmv, start=True, stop=True)
                    else:
                        nc.tensor.matmul(psA[0:64, off:off + N],
                                         qkT[0:64, A["fbk"], k0:k0 + 64],
                                         amv, start=True, stop=True)
                        nc.tensor.matmul(psB[64:128, off:off + N],
                                         qkT[64:128, Bc["fbk"], k0:k0 + 64],
                                         bmv, start=True, stop=True)
                        nc.tensor.matmul(psA[64:128, off:off + N],
                                         qkT[0:64, A["fbk"], k0 + 64:k0 + 128],
                                         amv, start=True, stop=True)
                        nc.tensor.matmul(psB[0:64, off:off + N],
                                         qkT[64:128, Bc["fbk"],
                                             k0 + 64:k0 + 128],
                                         bmv, start=True, stop=True)
                for ctx, ps, p in ((A, psA, pA), (Bc, psB, pB)):
                    nc.scalar.activation(p[:, 0:gw], ps[:, 0:gw],
                                         mybir.ActivationFunctionType.Exp,
                                         scale=0.125)
                    if g[0][0] >= A["n_full"]:
                        trit = tri if (NQ or ctx is A) else triB
                        for (kt, off, N, qoff) in g:
                            nc.vector.tensor_mul(p[:, off:off + 128],
                                                 p[:, off:off + 128], trit)
                    ctx["p"][gi] = p

            def emit_u(ctx, gi):
                if ctx["pu"] is None:
                    ctx["pu"] = psU.tile([128, 512], F32, tag="pu",
                                         name="pu_t")
                pu = ctx["pu"]
                p = ctx["p"][gi]
                lh = ctx["lh"]
                for (kt, off, N, qoff) in ctx["groups"][gi]:
                    nc.tensor.matmul(
                        pu[:, qoff:qoff + N],
                        v5[:, kt, lh >> 1, lh & 1, :],
                        p[:, off:off + N],
                        start=(kt == 0), stop=(kt == ctx["last_kt"]),
                        skip_group_check=True)

            def finalize(ctx):
                pu = ctx["pu"]
                po = ctx["po"]
                db = 64 - po             # denominator partitions
                rec = wp.tile([64, 512], F32, tag="rec", bufs=3, name="rec")
                if RSL:
                    nc.vector.reciprocal(rec[:, :], pu[db:db + 64, :])
                elif RFP:
                    nc.vector.reciprocal_approx_fast(rec[:, :],
                                                     pu[db:db + 64, :])
                else:
                    # copy to SBUF base-0 first: reciprocal_approx_fast
                    # (custom DVE op) NaNs when fed PSUM directly
                    den = wp.tile([64, 512], F32, tag="den", bufs=3,
                                  name="den")
                    nc.vector.tensor_copy(den[:, :], pu[db:db + 64, :])
                    nc.vector.reciprocal_approx_fast(rec[:, :], den[:, :])
                nc.vector.tensor_mul(
                    aT[po:po + 64, ctx["fbq"], ctx["qb"]:ctx["qb"] + 512],
                    pu[po:po + 64, :], rec[:, :])

            pending = []

            def flush_pending():
                while pending:
                    pending.pop(0)()

            def emit_pair(pr, qc, fills=()):
                A = head_ctx(2 * pr, qc)
                Bc = head_ctx(2 * pr + 1, qc)
                fills = list(fills)
                n = len(A["groups"])
                emit_scores_pair(A, Bc, 0)
                # previous pair's tail (last u group + finalize) overlaps
                # this pair's first scores/exp instead of stalling the PE
                flush_pending()
                for i in range(n):
                    if i + 1 < n:
                        emit_scores_pair(A, Bc, i + 1)
                    if fills:
                        emit_unit(fills.pop(0))
                    if i < n - 1:
                        emit_u(A, i)
                        emit_u(Bc, i)
                    else:
                        def tail(Ax=A, Bx=Bc, gi=i):
                            emit_u(Ax, gi)
                            emit_u(Bx, gi)
                            finalize(Ax)
                            finalize(Bx)
                        pending.append(tail)
                for u in fills:
                    emit_unit(u)

            # ---- stage 4: c_proj partial from a^T ----
            def emit_proj(st):
                for n0 in range(0, NX, 512):
                    ps = psU.tile([128, 512], F32, tag="pu")
                    for kt in range(FL // 128):
                        nc.tensor.matmul(ps[:], aT[:, kt, st * 128:(st + 1) * 128],
                                         wproj[:, kt, n0:n0 + 512],
                                         start=(kt == 0),
                                         stop=(NB and kt == FL // 128 - 1))
                    if not NB:
                        nc.tensor.matmul(ps[:], ones_row[:],
                                         bp_row[:, n0:n0 + 512],
                                         start=False, stop=True)
                    dst = out_ext.ap()[st * 128:(st + 1) * 128, n0:n0 + 512]
                    osb = wp.tile([128, 512], F32, tag="osb")
                    if st >= 12:
                        nc.scalar.copy(osb[:], ps[:])
                    else:
                        nc.vector.tensor_copy(osb[:], ps[:])
                    nc.sync.dma_start(out=dst, in_=osb[:])

            def emit_unit(u):
                kind = u[0]
                if kind == "v":
                    emit_v(u[1])
                elif kind == "qk":
                    emit_qk_half(u[1], u[2])
                else:
                    emit_proj(u[1])

            # ---- emission schedule: fully mixed stream ----
            # Pairs ordered so exp work reaches the scalar engine early
            # (it is the bottleneck in the attention phase); dense qkv/proj
            # units fill the PE while exps drain.  FB = FL//128 = 4.
            FB = FL // 128
            emit_qk_half(0, 0)
            emit_qk_half(FB, 0)
            for st in range(4):
                emit_v(st)
            emit_pair(0, 0, [("v", 4), ("v", 5)])
            emit_pair(0, 1, [("v", 6), ("v", 7), ("qk", 1, 0),
                             ("qk", FB + 1, 0)])
            emit_pair(1, 0, [("v", 8), ("v", 9)])
            emit_pair(1, 1, [("v", 10), ("v", 11), ("qk", 0, 1),
                             ("qk", FB, 1)])
            emit_pair(0, 2, [("qk", 2, 0), ("qk", FB + 2, 0), ("v", 12),
                             ("v", 13)])
            emit_pair(2, 0, [("v", 14), ("v", 15)])
            emit_pair(2, 1, [("qk", 3, 0), ("qk", FB + 3, 0), ("qk", 1, 1),
                             ("qk", FB + 1, 1)])
            emit_pair(1, 2)
            emit_pair(3, 0, [("qk", 2, 1), ("qk", FB + 2, 1)])
            emit_pair(3, 1, [("qk", 3, 1), ("qk", FB + 3, 1)])
            emit_pair(0, 3)
            emit_pair(2, 2, [("proj", 0), ("proj", 1)])
            emit_pair(1, 3, [("proj", 2), ("proj", 3)])
            emit_pair(3, 2, [("proj", 4), ("proj", 5)])
            emit_pair(2, 3, [("proj", 6), ("proj", 7)])
            emit_pair(3, 3, [("proj", 8), ("proj", 9), ("proj", 10),
                             ("proj", 11)])
            flush_pending()
            for st in range(12, NK):
                emit_proj(st)

    nc.compile()
    return nc


@functools.lru_cache(maxsize=2)
def _built(cfg: str):
    return _build(cfg)


def _in_maps(x, c_attn_w, c_attn_b, c_proj_w, c_proj_b):
    maps = []
    for core in range(N_CORES):
        b, hg = core // 2, core % 2
        f0 = hg * FL
        w_q = c_attn_w[:, f0:f0 + FL]
        w_k = c_attn_w[:, NX + f0:NX + f0 + FL]
        w_v = c_attn_w[:, 2 * NX + f0:2 * NX + f0 + FL]
        b_q = c_attn_b[f0:f0 + FL]
        b_k = c_attn_b[NX + f0:NX + f0 + FL]
        b_v = c_attn_b[2 * NX + f0:2 * NX + f0 + FL]
        maps.append({
            "xT": np.ascontiguousarray(x[b].T).astype(BF),
            "w_qk": np.concatenate([w_q, w_k], axis=1).astype(BF),
            "w_v": np.ascontiguousarray(w_v).astype(BF),
            "w_proj": np.ascontiguousarray(c_proj_w[f0:f0 + FL, :]).astype(BF),
            "b_qk": np.concatenate([b_q, b_k]).astype(np.float32).reshape(-1, 1),
            "bv_row": b_v.astype(BF).reshape(1, FL),
            "bp_row": (c_proj_b / 2.0).astype(BF).reshape(1, NX),
        })
    return maps


def _run(inputs, cfg=None, trace=False):
    if cfg is None:
        zero_bias = (not inputs["c_attn_b"].any()) and \
                    (not inputs["c_proj_b"].any())
        cfg = DEFAULT_CFG if zero_bias else DEFAULT_CFG_BIAS
    nc = _built(cfg)
    maps = _in_maps(inputs["x"], inputs["c_attn_w"], inputs["c_attn_b"],
                    inputs["c_proj_w"], inputs["c_proj_b"])
    res = run_bass_kernel_spmd(nc, maps, core_ids=list(range(N_CORES)),
                               trace=trace)
    out = np.empty((B, S, NX), dtype=np.float32)
    for b in range(B):
        out[b] = res.results[2 * b]["out"] + res.results[2 * b + 1]["out"]
    return out, res


def kernel(**inputs):
    out, _ = _run({k: np.asarray(v) for k, v in inputs.items()})
    return out
